# revision 14
# baseline (speedup 1.0000x reference)
"""Trainium2 Bass kernel for ConvAttentionCoefficients (GNN message passing).

out[e] = (x[idx_i[e]] @ Wq * w_ij[e] * x[idx_j[e]] @ Wk).sum(-1) / sqrt(F)

Strategy (8 NeuronCores, pure data-parallel over edges):
  - Replicate x, Wq, Wk on every core; shard the 640k edges 8 ways.
  - Phase 1 (replicated): project q = x @ (Wq/sqrt(F)), k = x @ Wk on-device
    (PE transpose + matmul), write bf16 q/k row tables to internal DRAM.
  - Phase 2 (sharded): dma_gather q[idx_i] / k[idx_j] rows in 1024-edge
    chunks (SWDGE descriptor-ring limit), stream bf16 w_ij, two bf16
    elementwise multiplies, segmented f32 reduce over F.

dma_gather needs int16 row indices but we have 40000 nodes.  The gather
base is therefore placed at table row 7232 and indices are biased by -7232
(range [-7232, 32767]); the SWDGE descriptor generator resolves negative
indices with plain signed address arithmetic (verified on HW).  Two
wrinkles, both handled here:
  - Rows 0..7232 sit outside the gather's declared AP, so Tile cannot see
    those write->read dependencies; a bounce-read of those rows (real RAW
    dep on the writes) plus explicit dep edges onto every gather enforces
    ordering.
  - Trailing negative indices in a chunk are treated as padding by the
    descriptor generator and dropped (mid-list negatives gather fine), so
    the host swaps an all-non-negative edge into every chunk's last slot
    and undoes the permutation on output.

dma_gather index operand layout: position g of a chunk's 1024 indices lives
at [g % 16, g // 16] of a [16, 64] int16 block, replicated 8x across the
128 partitions (one copy per GpSimd Q7 core).  Gathered row g lands in
partition g % 128, chunk g // 128, so chunk-local edge order is c*128 + p;
w_ij is read through a matching strided AP and the host un-permutes the
tiny per-edge output at the end.
"""

import math

import ml_dtypes
import numpy as np

import concourse.bacc as bacc
import concourse.bass as bass
import concourse.mybir as mybir
import concourse.tile as tile
from concourse.bass_utils import run_bass_kernel_spmd
from concourse.masks import make_identity
from concourse.tile import add_dep_helper

N_NODES = 40000
N_PAIRS = 640000
F = 128
N_CORES = 8
E_CORE = N_PAIRS // N_CORES  # 80000 edges per core

# Edge phase: gather chunk = 1024 edges (8 cols); compute tile = 3 chunks.
GC = 8  # cols per gather chunk
G_EDGES = 128 * GC  # 1024
CHUNKS_PER_TILE = 3
C = GC * CHUNKS_PER_TILE  # 24 cols per compute tile
E_TILE = 128 * C  # 3072
N_FULL_TILES = E_CORE // E_TILE  # 26
REM_EDGES = E_CORE - N_FULL_TILES * E_TILE  # 128
REM_COLS = REM_EDGES // 128  # 1
N_CHUNKS = N_FULL_TILES * CHUNKS_PER_TILE + 1  # 79 (last chunk = 128 idxs)
IDX_COLS = G_EDGES // 16  # 64

# Node-phase tiling: 40000 = 312*128 + 64; last tile overlaps by 64 rows.
NODE_TILE = 128
N_NTILES = math.ceil(N_NODES / NODE_TILE)  # 313
P1_GRP = 8  # node tiles per DMA group in phase 1

# Gather base sits at table row HI_ROWS; index bias -HI_ROWS keeps all
# 40000 rows inside int16 range [-7232, 32767].
LO_SPLIT = 32768
HI_ROWS = N_NODES - LO_SPLIT  # 7232

F32 = mybir.dt.float32
BF16 = mybir.dt.bfloat16
I16 = mybir.dt.int16
NP_BF16 = ml_dtypes.bfloat16

_CACHE = {}


def _build_nc(repeat=1):
    nc = bacc.Bacc(None, target_bir_lowering=False)

    x = nc.dram_tensor("x", [N_NODES, F], F32, kind="ExternalInput")
    w = nc.dram_tensor("w", [E_CORE, F], BF16, kind="ExternalInput")
    ii = nc.dram_tensor("ii", [N_CHUNKS, 128, IDX_COLS], I16, kind="ExternalInput")
    jj = nc.dram_tensor("jj", [N_CHUNKS, 128, IDX_COLS], I16, kind="ExternalInput")
    wq = nc.dram_tensor("wq", [F, F], F32, kind="ExternalInput")
    wk = nc.dram_tensor("wk", [F, F], F32, kind="ExternalInput")
    out = nc.dram_tensor("out", [E_CORE], F32, kind="ExternalOutput")

    qt = nc.dram_tensor("qt", [N_NODES, F], BF16, kind="Internal")
    kt = nc.dram_tensor("kt", [N_NODES, F], BF16, kind="Internal")

    inv_sqrt_f = 1.0 / math.sqrt(F)

    with tile.TileContext(nc) as tc:
        with (
            tc.tile_pool(name="const", bufs=1) as cpool,
            tc.tile_pool(name="p1", bufs=3) as p1,
            tc.tile_pool(name="psum", bufs=2, space="PSUM") as pp,
            tc.tile_pool(name="p2g", bufs=3) as p2g,
            tc.tile_pool(name="p2i", bufs=3) as p2i,
            tc.tile_pool(name="p2r", bufs=3) as p2r,
        ):
            ident = cpool.tile([128, 128], F32, tag="ident")
            make_identity(nc, ident[:])
            wq_sb = cpool.tile([F, F], F32, tag="wq")
            wk_sb = cpool.tile([F, F], F32, tag="wk")
            nc.sync.dma_start(wq_sb[:], wq[:])
            nc.sync.dma_start(wk_sb[:], wk[:])
            # Fold the 1/sqrt(F) epilogue scale into Wq.
            nc.vector.tensor_scalar_mul(wq_sb[:], wq_sb[:], inv_sqrt_f)

            for _rep in range(repeat):
                _build_body(nc, tc, cpool, p1, pp, p2g, p2i, p2r,
                            ident, wq_sb, wk_sb, x, w, ii, jj, out, qt, kt)

    nc.finalize()
    return nc


def _build_body(nc, tc, cpool, p1, pp, p2g, p2i, p2r,
                ident, wq_sb, wk_sb, x, w, ii, jj, out, qt, kt):
    if True:
        if True:
            # ---- Phase 1: q/k projection tables ----
            # Groups of GRP node tiles share one x load and one q/k store
            # each, to keep the SP engine's per-DMA issue overhead small.
            groups = []
            t = 0
            while t < N_NTILES:
                n = min(P1_GRP, N_NTILES - t)
                groups.append((t, n))
                t += n
            for g0, gn in groups:
                nb = min(g0 * NODE_TILE, N_NODES - NODE_TILE)
                ne = min((g0 + gn - 1) * NODE_TILE, N_NODES - NODE_TILE) + NODE_TILE
                nodes = ne - nb
                tiles = nodes // NODE_TILE
                xa = p1.tile([128, P1_GRP * F], F32, tag="xa")
                nc.sync.dma_start(
                    xa[:, : tiles * F].rearrange("p (t f) -> p t f", f=F),
                    x[nb:ne, :].rearrange("(t p) f -> p t f", p=128),
                )
                qs = p1.tile([128, P1_GRP * F], BF16, tag="qs")
                ks = p1.tile([128, P1_GRP * F], BF16, tag="ks")
                for ti in range(tiles):
                    sl = slice(ti * F, (ti + 1) * F)
                    xtp = pp.tile([128, 128], F32, tag="xtp")
                    nc.tensor.transpose(xtp[:], xa[:, sl], ident[:])
                    xt = p1.tile([128, F], F32, tag="xt")
                    nc.vector.tensor_copy(xt[:], xtp[:])
                    qp = pp.tile([128, F], F32, tag="qp")
                    nc.tensor.matmul(
                        qp[:], lhsT=xt[:], rhs=wq_sb[:], start=True, stop=True
                    )
                    kp = pp.tile([128, F], F32, tag="kp")
                    nc.tensor.matmul(
                        kp[:], lhsT=xt[:], rhs=wk_sb[:], start=True, stop=True
                    )
                    nc.scalar.copy(qs[:, sl], qp[:])
                    nc.scalar.copy(ks[:, sl], kp[:])
                nc.sync.dma_start(
                    qt[nb:ne, :].rearrange("(t p) f -> p t f", p=128),
                    qs[:, : tiles * F].rearrange("p (t f) -> p t f", f=F),
                )
                nc.sync.dma_start(
                    kt[nb:ne, :].rearrange("(t p) f -> p t f", p=128),
                    ks[:, : tiles * F].rearrange("p (t f) -> p t f", f=F),
                )

            # Rows 0..HI_ROWS are reached via negative gather indices,
            # outside the gathers' declared APs.  Bounce-read them: the read
            # carries a real RAW dep on the writes (so Tile inserts DMA
            # completion waits), and every gather gets a dep on the reads.
            bounce = cpool.tile([64, (HI_ROWS // 64) * F], BF16, tag="bounce")
            fences = [
                nc.sync.dma_start(
                    bounce[:].rearrange("b (a f) -> b a f", f=F),
                    qt[:HI_ROWS, :].rearrange("(a b) f -> b a f", b=64),
                ),
                nc.sync.dma_start(
                    bounce[:].rearrange("b (a f) -> b a f", f=F),
                    kt[:HI_ROWS, :].rearrange("(a b) f -> b a f", b=64),
                ),
            ]

            # All gather indices in two DMAs (SP issue overhead is per-DMA).
            iit_all = cpool.tile([128, N_CHUNKS * IDX_COLS], I16, tag="iit_all")
            nc.sync.dma_start(
                iit_all[:].rearrange("p (t c) -> p t c", c=IDX_COLS),
                ii[:].rearrange("t p c -> p t c"),
            )
            jjt_all = cpool.tile([128, N_CHUNKS * IDX_COLS], I16, tag="jjt_all")
            nc.sync.dma_start(
                jjt_all[:].rearrange("p (t c) -> p t c", c=IDX_COLS),
                jj[:].rearrange("t p c -> p t c"),
            )

            # ---- Phase 2: per-edge gather + reduce ----
            def edge_tile(t, cols, chunk0, nchunks):
                base = t * E_TILE
                wgt = p2g.tile([128, C * F], BF16, tag="wgt")
                nc.sync.dma_start(
                    wgt[:, : cols * F].rearrange("p (c f) -> p c f", f=F),
                    w[base : base + 128 * cols, :].rearrange(
                        "(c p) f -> p c f", p=128
                    ),
                )
                qg = p2g.tile([128, C * F], BF16, tag="qg")
                kg = p2g.tile([128, C * F], BF16, tag="kg")
                for s in range(nchunks):
                    ncols = min(GC, cols - s * GC)
                    nidx = 128 * ncols
                    ch = chunk0 + s
                    isl = slice(ch * IDX_COLS, ch * IDX_COLS + nidx // 16)
                    sl = slice(s * GC * F, (s * GC + ncols) * F)
                    gq = nc.gpsimd.dma_gather(
                        qg[:, sl].rearrange("p (c f) -> p c f", f=F),
                        qt[HI_ROWS:, :],
                        iit_all[:, isl],
                        num_idxs=nidx,
                        num_idxs_reg=nidx,
                        elem_size=F,
                    )
                    gk = nc.gpsimd.dma_gather(
                        kg[:, sl].rearrange("p (c f) -> p c f", f=F),
                        kt[HI_ROWS:, :],
                        jjt_all[:, isl],
                        num_idxs=nidx,
                        num_idxs_reg=nidx,
                        elem_size=F,
                    )
                    for fe in fences:
                        add_dep_helper(fe.ins, gq.ins, reason="gather after hi writes")
                        add_dep_helper(fe.ins, gk.ins, reason="gather after hi writes")
                n = cols * F
                nc.vector.tensor_mul(qg[:, :n], qg[:, :n], kg[:, :n])
                nc.vector.tensor_mul(qg[:, :n], qg[:, :n], wgt[:, :n])
                res = p2r.tile([128, C], F32, tag="res")
                nc.vector.tensor_reduce(
                    out=res[:, :cols].rearrange("p (c o) -> p c o", o=1),
                    in_=qg[:, :n].rearrange("p (c f) -> p c f", f=F),
                    axis=mybir.AxisListType.X,
                    op=mybir.AluOpType.add,
                )
                nc.sync.dma_start(
                    out[base : base + 128 * cols].rearrange("(p c) -> p c", p=128),
                    res[:, :cols],
                )

            for t in range(N_FULL_TILES):
                edge_tile(t, C, t * CHUNKS_PER_TILE, CHUNKS_PER_TILE)
            edge_tile(N_FULL_TILES, REM_COLS, N_FULL_TILES * CHUNKS_PER_TILE, 1)


def _get_nc(repeat=1):
    key = ("nc", repeat)
    if key not in _CACHE:
        _CACHE[key] = _build_nc(repeat)
    return _CACHE[key]


def _make_perm(bi, bj):
    """Per-core edge permutation (device-edge order): ensure the last slot
    of every gather chunk has both biased indices >= 0, so the descgen's
    trailing-negative truncation never fires."""
    perm = np.arange(E_CORE)
    for b, e in [(ch * G_EDGES, min(ch * G_EDGES + G_EDGES, E_CORE)) for ch in range(N_CHUNKS)]:
        tail = e - 1
        if bi[perm[tail]] < 0 or bj[perm[tail]] < 0:
            seg = perm[b:tail]
            good = np.where((bi[seg] >= 0) & (bj[seg] >= 0))[0]
            assert len(good), "no all-non-negative edge in chunk"
            g = b + good[-1]
            perm[tail], perm[g] = perm[g], perm[tail]
    return perm


def _pack_indices(idx16):
    """[E_CORE] biased int16 -> [N_CHUNKS, 128, IDX_COLS] dma_gather operand."""
    packed = np.zeros((N_CHUNKS, 16, IDX_COLS), np.int16)
    full = N_FULL_TILES * CHUNKS_PER_TILE * G_EDGES  # 79872
    packed[:-1] = idx16[:full].reshape(N_CHUNKS - 1, IDX_COLS, 16).transpose(0, 2, 1)
    rem = idx16[full:]
    packed[-1, :, : len(rem) // 16] = rem.reshape(len(rem) // 16, 16).T
    return np.ascontiguousarray(np.tile(packed, (1, 8, 1)))


def _unpermute(arr):
    """Device edge order within a tile is c*128 + p; undo it."""
    full = N_FULL_TILES * E_TILE
    head = arr[:full].reshape(N_FULL_TILES, 128, C).transpose(0, 2, 1).reshape(-1)
    tail = arr[full:].reshape(128, REM_COLS).T.reshape(-1)
    return np.concatenate([head, tail])


def make_in_maps(x, w_ij, idx_i, idx_j, Wq, Wk):
    x = np.ascontiguousarray(np.asarray(x, dtype=np.float32))
    w_ij = np.ascontiguousarray(np.asarray(w_ij).astype(NP_BF16))
    wq = np.ascontiguousarray(np.asarray(Wq, dtype=np.float32))
    wk = np.ascontiguousarray(np.asarray(Wk, dtype=np.float32))
    ii = np.asarray(idx_i, dtype=np.int64)
    jj = np.asarray(idx_j, dtype=np.int64)

    in_maps = []
    perms = []
    for c in range(N_CORES):
        sl = slice(c * E_CORE, (c + 1) * E_CORE)
        bi = (ii[sl] - HI_ROWS).astype(np.int32)
        bj = (jj[sl] - HI_ROWS).astype(np.int32)
        perm = _make_perm(bi, bj)
        perms.append(perm)
        in_maps.append(
            {
                "x": x,
                "w": np.ascontiguousarray(w_ij[sl][perm]),
                "ii": _pack_indices(bi[perm].astype(np.int16)),
                "jj": _pack_indices(bj[perm].astype(np.int16)),
                "wq": wq,
                "wk": wk,
            }
        )
    return in_maps, perms


def kernel(x, w_ij, idx_i, idx_j, Wq, Wk, **run_kwargs):
    nc = _get_nc()
    in_maps, perms = make_in_maps(x, w_ij, idx_i, idx_j, Wq, Wk)
    res = run_bass_kernel_spmd(
        nc, in_maps, core_ids=list(range(N_CORES)), **run_kwargs
    )
    outs = []
    for r, perm in zip(res.results, perms):
        dev = _unpermute(r["out"])
        o = np.empty(E_CORE, np.float32)
        o[perm] = dev
        outs.append(o)
    out = np.concatenate(outs)
    if run_kwargs:
        return out, res
    return out


# revision 15
# speedup vs baseline: 2.0241x; 2.0241x over previous
"""Trainium2 Bass kernel for ConvAttentionCoefficients (GNN message passing).

out[e] = (x[idx_i[e]] @ Wq * w_ij[e] * x[idx_j[e]] @ Wk).sum(-1) / sqrt(F)

Strategy (8 NeuronCores, pure data-parallel over edges):
  - Replicate x, Wq, Wk on every core; shard the 640k edges 8 ways.
  - Phase 1 (replicated): project q = x @ (Wq/sqrt(F)), k = x @ Wk on-device
    (PE transpose + matmul), write bf16 q/k row tables to internal DRAM.
  - Phase 2 (sharded): dma_gather q[idx_i] / k[idx_j] rows in 1024-edge
    chunks (SWDGE descriptor-ring limit), stream bf16 w_ij, two bf16
    elementwise multiplies, segmented f32 reduce over F.

dma_gather needs int16 row indices but we have 40000 nodes.  The gather
base is therefore placed at table row 7232 and indices are biased by -7232
(range [-7232, 32767]); the SWDGE descriptor generator resolves negative
indices with plain signed address arithmetic (verified on HW).  Two
wrinkles, both handled here:
  - Rows 0..7232 sit outside the gather's declared AP, so Tile cannot see
    those write->read dependencies; a bounce-read of those rows (real RAW
    dep on the writes) plus explicit dep edges onto every gather enforces
    ordering.
  - Trailing negative indices in a chunk are treated as padding by the
    descriptor generator and dropped (mid-list negatives gather fine), so
    the host swaps an all-non-negative edge into every chunk's last slot
    and undoes the permutation on output.

dma_gather index operand layout: position g of a chunk's 1024 indices lives
at [g % 16, g // 16] of a [16, 64] int16 block, replicated 8x across the
128 partitions (one copy per GpSimd Q7 core).  Gathered row g lands in
partition g % 128, chunk g // 128, so chunk-local edge order is c*128 + p;
w_ij is read through a matching strided AP and the host un-permutes the
tiny per-edge output at the end.
"""

import math

import ml_dtypes
import numpy as np

import concourse.bacc as bacc
import concourse.bass as bass
import concourse.mybir as mybir
import concourse.tile as tile
from concourse.bass_utils import run_bass_kernel_spmd
from concourse.masks import make_identity
from concourse.tile import add_dep_helper

N_NODES = 40000
N_PAIRS = 640000
F = 128
N_CORES = 8
E_CORE = N_PAIRS // N_CORES  # 80000 edges per core

# Edge phase: gather chunk = 1024 edges (8 cols); compute tile = 3 chunks.
GC = 8  # cols per gather chunk
G_EDGES = 128 * GC  # 1024
CHUNKS_PER_TILE = 3
C = GC * CHUNKS_PER_TILE  # 24 cols per compute tile
E_TILE = 128 * C  # 3072
N_FULL_TILES = E_CORE // E_TILE  # 26
REM_EDGES = E_CORE - N_FULL_TILES * E_TILE  # 128
REM_COLS = REM_EDGES // 128  # 1
N_CHUNKS = N_FULL_TILES * CHUNKS_PER_TILE + 1  # 79 (last chunk = 128 idxs)
IDX_COLS = G_EDGES // 16  # 64

# Node-phase tiling: 40000 = 312*128 + 64; last tile overlaps by 64 rows.
NODE_TILE = 128
N_NTILES = math.ceil(N_NODES / NODE_TILE)  # 313
P1_GRP = 8  # node tiles per DMA group in phase 1

# Gather base sits at table row HI_ROWS; index bias -HI_ROWS keeps all
# 40000 rows inside int16 range [-7232, 32767].
LO_SPLIT = 32768
HI_ROWS = N_NODES - LO_SPLIT  # 7232

F32 = mybir.dt.float32
BF16 = mybir.dt.bfloat16
I16 = mybir.dt.int16
NP_BF16 = ml_dtypes.bfloat16

_CACHE = {}


def _build_nc(repeat=1):
    nc = bacc.Bacc(None, target_bir_lowering=False, num_swdge_queues=4)

    x = nc.dram_tensor("x", [N_NODES, F], F32, kind="ExternalInput")
    w = nc.dram_tensor("w", [E_CORE, F], BF16, kind="ExternalInput")
    ii = nc.dram_tensor("ii", [N_CHUNKS, 128, IDX_COLS], I16, kind="ExternalInput")
    jj = nc.dram_tensor("jj", [N_CHUNKS, 128, IDX_COLS], I16, kind="ExternalInput")
    wq = nc.dram_tensor("wq", [F, F], F32, kind="ExternalInput")
    wk = nc.dram_tensor("wk", [F, F], F32, kind="ExternalInput")
    out = nc.dram_tensor("out", [E_CORE], F32, kind="ExternalOutput")

    qt = nc.dram_tensor("qt", [N_NODES, F], BF16, kind="Internal")
    kt = nc.dram_tensor("kt", [N_NODES, F], BF16, kind="Internal")

    inv_sqrt_f = 1.0 / math.sqrt(F)

    with tile.TileContext(nc) as tc:
        with (
            tc.tile_pool(name="const", bufs=1) as cpool,
            tc.tile_pool(name="p1", bufs=3) as p1,
            tc.tile_pool(name="psum", bufs=2, space="PSUM") as pp,
            tc.tile_pool(name="p2g", bufs=3) as p2g,
            tc.tile_pool(name="p2i", bufs=3) as p2i,
            tc.tile_pool(name="p2r", bufs=3) as p2r,
        ):
            ident = cpool.tile([128, 128], F32, tag="ident")
            make_identity(nc, ident[:])
            wq_sb = cpool.tile([F, F], F32, tag="wq")
            wk_sb = cpool.tile([F, F], F32, tag="wk")
            nc.sync.dma_start(wq_sb[:], wq[:])
            nc.sync.dma_start(wk_sb[:], wk[:])
            # Fold the 1/sqrt(F) epilogue scale into Wq.
            nc.vector.tensor_scalar_mul(wq_sb[:], wq_sb[:], inv_sqrt_f)

            for _rep in range(repeat):
                _build_body(nc, tc, cpool, p1, pp, p2g, p2i, p2r,
                            ident, wq_sb, wk_sb, x, w, ii, jj, out, qt, kt)

    nc.finalize()
    return nc


def _build_body(nc, tc, cpool, p1, pp, p2g, p2i, p2r,
                ident, wq_sb, wk_sb, x, w, ii, jj, out, qt, kt):
    if True:
        if True:
            # ---- Phase 1: q/k projection tables ----
            # Groups of GRP node tiles share one x load and one q/k store
            # each, to keep the SP engine's per-DMA issue overhead small.
            groups = []
            t = 0
            while t < N_NTILES:
                n = min(P1_GRP, N_NTILES - t)
                groups.append((t, n))
                t += n
            for g0, gn in groups:
                nb = min(g0 * NODE_TILE, N_NODES - NODE_TILE)
                ne = min((g0 + gn - 1) * NODE_TILE, N_NODES - NODE_TILE) + NODE_TILE
                nodes = ne - nb
                tiles = nodes // NODE_TILE
                xa = p1.tile([128, P1_GRP * F], F32, tag="xa")
                nc.sync.dma_start(
                    xa[:, : tiles * F].rearrange("p (t f) -> p t f", f=F),
                    x[nb:ne, :].rearrange("(t p) f -> p t f", p=128),
                )
                qs = p1.tile([128, P1_GRP * F], BF16, tag="qs")
                ks = p1.tile([128, P1_GRP * F], BF16, tag="ks")
                for ti in range(tiles):
                    sl = slice(ti * F, (ti + 1) * F)
                    xtp = pp.tile([128, 128], F32, tag="xtp")
                    nc.tensor.transpose(xtp[:], xa[:, sl], ident[:])
                    xt = p1.tile([128, F], F32, tag="xt")
                    nc.vector.tensor_copy(xt[:], xtp[:])
                    qp = pp.tile([128, F], F32, tag="qp")
                    nc.tensor.matmul(
                        qp[:], lhsT=xt[:], rhs=wq_sb[:], start=True, stop=True
                    )
                    kp = pp.tile([128, F], F32, tag="kp")
                    nc.tensor.matmul(
                        kp[:], lhsT=xt[:], rhs=wk_sb[:], start=True, stop=True
                    )
                    nc.scalar.copy(qs[:, sl], qp[:])
                    nc.scalar.copy(ks[:, sl], kp[:])
                nc.sync.dma_start(
                    qt[nb:ne, :].rearrange("(t p) f -> p t f", p=128),
                    qs[:, : tiles * F].rearrange("p (t f) -> p t f", f=F),
                )
                nc.sync.dma_start(
                    kt[nb:ne, :].rearrange("(t p) f -> p t f", p=128),
                    ks[:, : tiles * F].rearrange("p (t f) -> p t f", f=F),
                )

            # Rows 0..HI_ROWS are reached via negative gather indices,
            # outside the gathers' declared APs.  Bounce-read them: the read
            # carries a real RAW dep on the writes (so Tile inserts DMA
            # completion waits), and every gather gets a dep on the reads.
            bounce = cpool.tile([64, (HI_ROWS // 64) * F], BF16, tag="bounce")
            fences = [
                nc.sync.dma_start(
                    bounce[:].rearrange("b (a f) -> b a f", f=F),
                    qt[:HI_ROWS, :].rearrange("(a b) f -> b a f", b=64),
                ),
                nc.sync.dma_start(
                    bounce[:].rearrange("b (a f) -> b a f", f=F),
                    kt[:HI_ROWS, :].rearrange("(a b) f -> b a f", b=64),
                ),
            ]

            # All gather indices in two DMAs (SP issue overhead is per-DMA).
            iit_all = cpool.tile([128, N_CHUNKS * IDX_COLS], I16, tag="iit_all")
            nc.sync.dma_start(
                iit_all[:].rearrange("p (t c) -> p t c", c=IDX_COLS),
                ii[:].rearrange("t p c -> p t c"),
            )
            jjt_all = cpool.tile([128, N_CHUNKS * IDX_COLS], I16, tag="jjt_all")
            nc.sync.dma_start(
                jjt_all[:].rearrange("p (t c) -> p t c", c=IDX_COLS),
                jj[:].rearrange("t p c -> p t c"),
            )

            # ---- Phase 2: per-edge gather + reduce ----
            # Round-robin gathers over the 4 SWDGE queues: each queue is
            # served by its own GpSimd Q7 core pair, so descriptor
            # generation (the serial cost of dma_gather) runs 4-wide.
            qn_counter = [0]

            def edge_tile(t, cols, chunk0, nchunks):
                base = t * E_TILE
                wgt = p2g.tile([128, C * F], BF16, tag="wgt")
                nc.sync.dma_start(
                    wgt[:, : cols * F].rearrange("p (c f) -> p c f", f=F),
                    w[base : base + 128 * cols, :].rearrange(
                        "(c p) f -> p c f", p=128
                    ),
                )
                qg = p2g.tile([128, C * F], BF16, tag="qg")
                kg = p2g.tile([128, C * F], BF16, tag="kg")
                for s in range(nchunks):
                    ncols = min(GC, cols - s * GC)
                    nidx = 128 * ncols
                    ch = chunk0 + s
                    isl = slice(ch * IDX_COLS, ch * IDX_COLS + nidx // 16)
                    sl = slice(s * GC * F, (s * GC + ncols) * F)
                    gq = nc.gpsimd.dma_gather(
                        qg[:, sl].rearrange("p (c f) -> p c f", f=F),
                        qt[HI_ROWS:, :],
                        iit_all[:, isl],
                        num_idxs=nidx,
                        num_idxs_reg=nidx,
                        elem_size=F,
                        queue_num=qn_counter[0] % 4,
                    )
                    gk = nc.gpsimd.dma_gather(
                        kg[:, sl].rearrange("p (c f) -> p c f", f=F),
                        kt[HI_ROWS:, :],
                        jjt_all[:, isl],
                        num_idxs=nidx,
                        num_idxs_reg=nidx,
                        elem_size=F,
                        queue_num=(qn_counter[0] + 1) % 4,
                    )
                    qn_counter[0] += 2
                    for fe in fences:
                        add_dep_helper(fe.ins, gq.ins, reason="gather after hi writes")
                        add_dep_helper(fe.ins, gk.ins, reason="gather after hi writes")
                n = cols * F
                nc.vector.tensor_mul(qg[:, :n], qg[:, :n], kg[:, :n])
                nc.vector.tensor_mul(qg[:, :n], qg[:, :n], wgt[:, :n])
                res = p2r.tile([128, C], F32, tag="res")
                nc.vector.tensor_reduce(
                    out=res[:, :cols].rearrange("p (c o) -> p c o", o=1),
                    in_=qg[:, :n].rearrange("p (c f) -> p c f", f=F),
                    axis=mybir.AxisListType.X,
                    op=mybir.AluOpType.add,
                )
                nc.sync.dma_start(
                    out[base : base + 128 * cols].rearrange("(p c) -> p c", p=128),
                    res[:, :cols],
                )

            for t in range(N_FULL_TILES):
                edge_tile(t, C, t * CHUNKS_PER_TILE, CHUNKS_PER_TILE)
            edge_tile(N_FULL_TILES, REM_COLS, N_FULL_TILES * CHUNKS_PER_TILE, 1)


def _get_nc(repeat=1):
    key = ("nc", repeat)
    if key not in _CACHE:
        _CACHE[key] = _build_nc(repeat)
    return _CACHE[key]


def _make_perm(bi, bj):
    """Per-core edge permutation (device-edge order): ensure the last slot
    of every gather chunk has both biased indices >= 0, so the descgen's
    trailing-negative truncation never fires."""
    perm = np.arange(E_CORE)
    for b, e in [(ch * G_EDGES, min(ch * G_EDGES + G_EDGES, E_CORE)) for ch in range(N_CHUNKS)]:
        tail = e - 1
        if bi[perm[tail]] < 0 or bj[perm[tail]] < 0:
            seg = perm[b:tail]
            good = np.where((bi[seg] >= 0) & (bj[seg] >= 0))[0]
            assert len(good), "no all-non-negative edge in chunk"
            g = b + good[-1]
            perm[tail], perm[g] = perm[g], perm[tail]
    return perm


def _pack_indices(idx16):
    """[E_CORE] biased int16 -> [N_CHUNKS, 128, IDX_COLS] dma_gather operand."""
    packed = np.zeros((N_CHUNKS, 16, IDX_COLS), np.int16)
    full = N_FULL_TILES * CHUNKS_PER_TILE * G_EDGES  # 79872
    packed[:-1] = idx16[:full].reshape(N_CHUNKS - 1, IDX_COLS, 16).transpose(0, 2, 1)
    rem = idx16[full:]
    packed[-1, :, : len(rem) // 16] = rem.reshape(len(rem) // 16, 16).T
    return np.ascontiguousarray(np.tile(packed, (1, 8, 1)))


def _unpermute(arr):
    """Device edge order within a tile is c*128 + p; undo it."""
    full = N_FULL_TILES * E_TILE
    head = arr[:full].reshape(N_FULL_TILES, 128, C).transpose(0, 2, 1).reshape(-1)
    tail = arr[full:].reshape(128, REM_COLS).T.reshape(-1)
    return np.concatenate([head, tail])


def make_in_maps(x, w_ij, idx_i, idx_j, Wq, Wk):
    x = np.ascontiguousarray(np.asarray(x, dtype=np.float32))
    w_ij = np.ascontiguousarray(np.asarray(w_ij).astype(NP_BF16))
    wq = np.ascontiguousarray(np.asarray(Wq, dtype=np.float32))
    wk = np.ascontiguousarray(np.asarray(Wk, dtype=np.float32))
    ii = np.asarray(idx_i, dtype=np.int64)
    jj = np.asarray(idx_j, dtype=np.int64)

    in_maps = []
    perms = []
    for c in range(N_CORES):
        sl = slice(c * E_CORE, (c + 1) * E_CORE)
        bi = (ii[sl] - HI_ROWS).astype(np.int32)
        bj = (jj[sl] - HI_ROWS).astype(np.int32)
        perm = _make_perm(bi, bj)
        perms.append(perm)
        in_maps.append(
            {
                "x": x,
                "w": np.ascontiguousarray(w_ij[sl][perm]),
                "ii": _pack_indices(bi[perm].astype(np.int16)),
                "jj": _pack_indices(bj[perm].astype(np.int16)),
                "wq": wq,
                "wk": wk,
            }
        )
    return in_maps, perms


def kernel(x, w_ij, idx_i, idx_j, Wq, Wk, **run_kwargs):
    nc = _get_nc()
    in_maps, perms = make_in_maps(x, w_ij, idx_i, idx_j, Wq, Wk)
    res = run_bass_kernel_spmd(
        nc, in_maps, core_ids=list(range(N_CORES)), **run_kwargs
    )
    outs = []
    for r, perm in zip(res.results, perms):
        dev = _unpermute(r["out"])
        o = np.empty(E_CORE, np.float32)
        o[perm] = dev
        outs.append(o)
    out = np.concatenate(outs)
    if run_kwargs:
        return out, res
    return out
